# revision 6
# baseline (speedup 1.0000x reference)
"""Trainium2 Bass kernel for nn_MultiHeadAttention (B=4, S=2048, D=1024, H=16).

Sharding: 8 cores = 4 batches x 2 query-halves. Each core computes full K/V
projections for its batch (keys are permuted so the core's own queries come
first), attention for its 1024 queries over all 2048 keys, and the output
projection for its query half. No collectives needed.

Per-core dataflow (bf16 matmuls, fp32 PSUM accumulation, all tensors SBUF
resident):
  x [2048,1024] bf16 --PE transpose--> xT [D,S], streamed just-in-time so
  the first scores matmul issues ~9us in.
  qT = (x @ Wq + bq)^T per pair; kT = (x @ Wk)^T per pair (bk dropped: the
  q.bk score offset is constant over keys and cancels in softmax).
  v  = x @ Wv (no bias: bv contributes bv@Wo to y since sum(attn)=1, folded
  into bo on the host).
  Per head pair p, per q-span of 512:
    scoresT[k,q] via row-paired (tile_position) K=64 matmuls
    exp on ACT (scale=1/8 folded in), no max subtraction
    attnV: two col-tiled M=64 matmuls (cols 0-63 / 64-127) run concurrently,
    accumulating o_pair[128,512] over 16 k-tiles in PSUM.
    softmax denominator on DVE: partition-halving adds 128->64->32 (bf16),
    accumulated fp32 over k-tiles; finished with a small tree + fast
    reciprocal, broadcast via two col-tiled K=1 matmuls, one fused
    normalize-multiply writes oT.
  y = oT^T @ Wo + bo'  (bo' = bo + bv@Wo precomputed on host)
  O-projection for span 0 interleaved into pair 7 span 1 to shorten the
  tail.
"""

import numpy as np
import ml_dtypes
from contextlib import ExitStack

import concourse.bass as bass
from concourse import bacc
import concourse.mybir as mybir
import concourse.tile as tile
from concourse.bass_utils import run_bass_kernel_spmd
from concourse.masks import make_identity

F32 = mybir.dt.float32
BF16 = mybir.dt.bfloat16
AF = mybir.ActivationFunctionType
NPBF16 = ml_dtypes.bfloat16

P = 128

N_CORES = 8
B_FULL, S_FULL, D_FULL = 4, 2048, 1024
H_FULL, DH = 16, 64


def build_mha_nc(S=2048, Sq=1024, D=1024, H=16, scale=None):
    """Build the per-core Bass program. Returns nc."""
    assert D % P == 0 and S % P == 0 and Sq % P == 0 and H % 2 == 0
    ND = D // P            # d-tiles
    NS = S // P            # s-chunks / k-tiles
    NPAIR = H // 2
    QSP = min(512, Sq)     # q span
    NQS = Sq // QSP
    KSP = min(512, S)      # span for kT projection
    NKS = S // KSP
    CSP = min(512, D)      # col span for v / out projections
    NCS = D // CSP
    HPS = CSP // DH        # heads per col-span in v projection
    if scale is None:
        scale = DH ** -0.5

    nc = bacc.Bacc(target_bir_lowering=False, debug=False)

    x = nc.dram_tensor("x", [S, D], BF16, kind="ExternalInput").ap()
    W = {n: nc.dram_tensor(n, [D, D], BF16, kind="ExternalInput").ap()
         for n in ("Wq", "Wk", "Wv", "Wo")}
    bias = {n: nc.dram_tensor(n, [D], F32, kind="ExternalInput").ap()
            for n in ("bq", "bo")}
    y = nc.dram_tensor("y", [Sq, D], F32, kind="ExternalOutput").ap()

    with tile.TileContext(nc) as tc, ExitStack() as top:
        top.enter_context(nc.allow_low_precision(
            reason="bf16 activations/weights with fp32 psum accumulation"))
        const = top.enter_context(tc.tile_pool(name="const", bufs=1))
        big = top.enter_context(tc.tile_pool(name="big", bufs=1))
        wp = top.enter_context(tc.tile_pool(name="wp", bufs=2))
        kpool = top.enter_context(tc.tile_pool(name="kpool", bufs=3))

        ident = const.tile([P, P], BF16)
        make_identity(nc, ident)
        # bf16 ones row: K=1 stationary broadcasting the softmax reciprocal
        ones_t = const.tile([1, DH], BF16)
        nc.vector.memset(ones_t, 1.0)
        # warm the ACT exp table while DMAs run
        warm = const.tile([1, 2], BF16)
        nc.scalar.activation(warm, ones_t[:, 0:2], AF.Exp, scale=1.0)

        # per-partition bias layouts: b_sb[p, j] = b[j*128 + p]
        bq_sb = const.tile([P, ND], F32)
        nc.gpsimd.dma_start(out=bq_sb, in_=bias["bq"].rearrange("(j p) -> p j", p=P))
        # bo broadcast across partitions (0-stride DRAM read)
        bo_bc = const.tile([P, D], F32)
        nc.gpsimd.dma_start(
            out=bo_bc,
            in_=bias["bo"].unsqueeze(0).partition_broadcast(P).squeeze(1),
        )

        oT = big.tile([P, ND, Sq], BF16)
        xT = big.tile([P, ND, S], BF16)
        qTs = big.tile([P, ND, Sq], BF16)
        v_sb = big.tile([P, NS, H * DH], BF16)
        v3 = v_sb.rearrange("p i (h w) -> p i h w", w=DH)

        # weight staging (wp "w" rotates: Wk, Wq, then Wo; Wv pinned)
        Wk_sb = wp.tile([P, ND, D], BF16, tag="w", name="Wk")
        nc.sync.dma_start(out=Wk_sb, in_=W["Wk"].rearrange("(j p) c -> p j c", p=P))
        Wq_sb = wp.tile([P, ND, D], BF16, tag="w", name="Wq")
        nc.sync.dma_start(out=Wq_sb, in_=W["Wq"].rearrange("(j p) c -> p j c", p=P))
        Wv_sb = wp.tile([P, ND, D], BF16, tag="wv", bufs=1)
        nc.sync.dma_start(out=Wv_sb, in_=W["Wv"].rearrange("(j p) c -> p j c", p=P))

        kps = {}
        wo_box = {}

        with tc.tile_pool(name="xchunk", bufs=3) as xpool, \
             tc.tile_pool(name="tps", bufs=1, space="PSUM") as tpsum, \
             tc.tile_pool(name="pps", bufs=2, space="PSUM") as pps, \
             tc.tile_pool(name="exp", bufs=4) as exq, \
             tc.tile_pool(name="eps", bufs=1) as eps, \
             tc.tile_pool(name="dacc", bufs=1) as dacc, \
             tc.tile_pool(name="scps", bufs=2, space="PSUM") as scps, \
             tc.tile_pool(name="ops", bufs=1, space="PSUM") as opsum, \
             tc.tile_pool(name="ystg", bufs=2) as ystg:

            # single PSUM bank for transposes, manually double-buffered
            tps2 = tpsum.tile([P, 2, P], BF16)

            def transp(i):
                xc = xpool.tile([P, D], BF16, tag="xc", name=f"xc_{i}")
                nc.sync.dma_start(out=xc, in_=x[i * P:(i + 1) * P, :])
                for j in range(ND):
                    tp = tps2[:, (i * ND + j) % 2, :]
                    nc.tensor.transpose(tp, xc[:, j * P:(j + 1) * P], ident)
                    nc.vector.tensor_copy(xT[:, j, i * P:(i + 1) * P], tp)

            def qT_proj(dc, sp):
                ps = pps.tile([P, QSP], F32, tag="pp", name=f"qps_{dc}_{sp}")
                for j in range(ND):
                    nc.tensor.matmul(
                        ps,
                        Wq_sb[:, j, dc * P:(dc + 1) * P],
                        xT[:, j, sp * QSP:(sp + 1) * QSP],
                        start=(j == 0), stop=(j == ND - 1),
                    )
                nc.vector.tensor_scalar_add(
                    qTs[:, dc, sp * QSP:(sp + 1) * QSP], ps, bq_sb[:, dc:dc + 1])

            def v_proj(i, sp):
                ps = pps.tile([P, CSP], F32, tag="pp", name=f"vps_{i}_{sp}")
                for j in range(ND):
                    nc.tensor.matmul(
                        ps,
                        xT[:, j, i * P:(i + 1) * P],
                        Wv_sb[:, j, sp * CSP:(sp + 1) * CSP],
                        start=(j == 0), stop=(j == ND - 1),
                    )
                nc.vector.tensor_copy(
                    v3[:, i, sp * HPS:(sp + 1) * HPS, 0:DH],
                    ps.rearrange("p (h w) -> p h w", w=DH),
                )

            def kT_span(p, sp):
                kp = kps[p]
                ps = pps.tile([P, KSP], F32, tag="pp", name=f"kps_{p}_{sp}")
                for j in range(ND):
                    nc.tensor.matmul(
                        ps,
                        Wk_sb[:, j, p * P:(p + 1) * P],
                        xT[:, j, sp * KSP:(sp + 1) * KSP],
                        start=(j == 0), stop=(j == ND - 1),
                    )
                nc.vector.tensor_copy(kp[:, sp * KSP:(sp + 1) * KSP], ps)

            def new_kp(p):
                kps[p] = kpool.tile([P, S], BF16, tag="kp", name=f"kp_{p}")

            def load_wo():
                Wo_sb = wp.tile([P, ND, D], BF16, tag="w", name="Wo")
                nc.sync.dma_start(
                    out=Wo_sb, in_=W["Wo"].rearrange("(j p) c -> p j c", p=P))
                wo_box["Wo"] = Wo_sb

            def o_chunk(sc_i, spc):
                Wo_sb = wo_box["Wo"]
                ps = pps.tile([P, CSP], F32, tag="pp", name=f"yps_{sc_i}_{spc}")
                for j in range(ND):
                    nc.tensor.matmul(
                        ps,
                        oT[:, j, sc_i * P:(sc_i + 1) * P],
                        Wo_sb[:, j, spc * CSP:(spc + 1) * CSP],
                        start=(j == 0), stop=(j == ND - 1),
                    )
                ysb = ystg.tile([P, CSP], F32, tag="ysb")
                nc.vector.tensor_add(ysb, ps, bo_bc[:, spc * CSP:(spc + 1) * CSP])
                nc.sync.dma_start(
                    out=y[sc_i * P:(sc_i + 1) * P, spc * CSP:(spc + 1) * CSP],
                    in_=ysb,
                )

            # ---- slim prologue: just enough for pair 0 span 0 kt 0-3 ----
            for i in range(4):
                transp(i)
            new_kp(0)
            kT_span(0, 0)
            qT_proj(0, 0)

            # deferred-work schedule: (pair, span, kt) -> [thunks]
            jobs = {}

            def add(p, sp, kt, fn):
                jobs.setdefault((p, sp, kt), []).append(fn)

            # pair 0, span 0: remaining transposes + kp(0) spans (v is jit)
            add(0, 0, 1, lambda: (transp(4), transp(5)))
            add(0, 0, 2, lambda: (transp(6), transp(7)))
            add(0, 0, 3, lambda: kT_span(0, 1))
            add(0, 0, 4, lambda: (transp(8), transp(9)))
            add(0, 0, 5, lambda: (transp(10), transp(11)))
            add(0, 0, 6, lambda: kT_span(0, 2))
            add(0, 0, 7, lambda: (transp(12), transp(13)))
            add(0, 0, 8, lambda: (transp(14), transp(15)))
            add(0, 0, 9, lambda: kT_span(0, 3))

            # pair 0, span 1: qT(0,1), kp(1), qT(1)
            add(0, 1, 0, lambda: qT_proj(0, 1))
            add(0, 1, 1, lambda: new_kp(1))
            for sp_ in range(NKS):
                add(0, 1, 2 + sp_ * 2, lambda sp_=sp_: kT_span(1, sp_))
            add(0, 1, 11, lambda: qT_proj(1, 0))
            add(0, 1, 13, lambda: qT_proj(1, 1))

            # pairs 1..6: kp(p+1) in span 0, qT(p+1) in span 1
            for p_ in range(1, NPAIR - 1):
                add(p_, 0, 0, lambda p_=p_: new_kp(p_ + 1))
                for sp_ in range(NKS):
                    add(p_, 0, 1 + sp_ * 3,
                        lambda p_=p_, sp_=sp_: kT_span(p_ + 1, sp_))
                add(p_, 1, 1, lambda p_=p_: qT_proj(p_ + 1, 0))
                add(p_, 1, 8, lambda p_=p_: qT_proj(p_ + 1, 1))
            # v sp1 (heads 8-15, needed from pair 4): 16 chains over pairs 1-3
            vslots = [(p_, sp_, kt_) for p_ in (1, 2, 3)
                      for sp_ in (0, 1) for kt_ in (2, 6, 12)]
            for i_ in range(NS):
                p_, sp_, kt_ = vslots[i_]
                add(p_, sp_, kt_, lambda i_=i_: v_proj(i_, 1))
            # Wo load + O-projection span 0 interleaved into pair 7
            add(NPAIR - 2, 1, 14, load_wo)
            for ci, (sc_i, spc) in enumerate(
                    [(si, c) for si in range(QSP // P) for c in range(NCS)]):
                add(NPAIR - 1, 1, 2 * ci, lambda a=sc_i, b=spc: o_chunk(a, b))

            # ---- attention: pair-outer, span-inner ----
            for p in range(NPAIR):
                for sp in range(NQS):
                    qsl = slice(sp * QSP, (sp + 1) * QSP)
                    kp = kps[p]
                    o_pair = opsum.tile([P, QSP], F32, tag="op")
                    den64 = dacc.tile([DH, 2 * QSP], F32, tag="dn")
                    for kt in range(NS):
                        for fn in jobs.get((p, sp, kt), ()):
                            fn()
                        sc = scps.tile([P, 2 * QSP], F32, tag="sc")
                        nc.tensor.matmul(
                            sc[:, 0:QSP],
                            kp[0:DH, kt * P:(kt + 1) * P],
                            qTs[0:DH, p, qsl],
                            start=True, stop=True,
                        )
                        nc.tensor.matmul(
                            sc[:, QSP:2 * QSP],
                            kp[DH:P, kt * P:(kt + 1) * P],
                            qTs[DH:P, p, qsl],
                            start=True, stop=True,
                        )
                        ex = exq.tile([P, 2 * QSP], BF16, tag="ex")
                        nc.scalar.activation(ex, sc, AF.Exp, scale=float(scale))
                        if p == 0 and sp == 0:
                            v_proj(kt, 0)
                        nc.tensor.matmul(
                            o_pair[0:DH, :],
                            v3[:, kt, 2 * p, :],
                            ex[:, 0:QSP],
                            start=(kt == 0), stop=(kt == NS - 1),
                        )
                        nc.tensor.matmul(
                            o_pair[DH:P, :],
                            v3[:, kt, 2 * p + 1, :],
                            ex[:, QSP:2 * QSP],
                            start=(kt == 0), stop=(kt == NS - 1),
                        )
                        # denominator on DVE: fold 128 -> 64 in bf16 (copy
                        # realigns the base partition: tensor_tensor requires
                        # equal SBUF base partitions), then fp32 accumulate
                        hi = eps.tile([DH, 2 * QSP], BF16, tag="hi")
                        nc.vector.tensor_copy(hi, ex[DH:P, :])
                        nc.vector.tensor_add(hi, ex[0:DH, :], hi)
                        if kt == 0:
                            nc.vector.tensor_copy(den64, hi)
                        else:
                            nc.vector.tensor_add(den64, den64, hi)
                    # epilogue: partition tree 64 -> 1. SBUF partition bases
                    # must be 32-aligned, so 64->32 uses a base-aligned copy
                    # and the sub-32 levels use stream_shuffle (out[i] <-
                    # in[mask[i]], 255 = no write) to realign operands.
                    tsc = eps.tile([32, 2 * QSP], F32, tag="tsc")
                    nc.vector.tensor_copy(tsc, den64[32:DH, :])
                    nc.vector.tensor_add(den64[0:32, :], den64[0:32, :], tsc)
                    n = 16
                    while n >= 1:
                        mask = list(range(n, 2 * n)) + [255] * (32 - n)
                        nc.vector.stream_shuffle(
                            tsc, den64[0:32, :], mask)
                        nc.vector.tensor_add(
                            den64[0:n, :], den64[0:n, :], tsc[0:n, :])
                        n //= 2
                    rc = eps.tile([1, 2 * QSP], F32, tag="rc")
                    nc.vector.reciprocal_approx_fast(rc, den64[0:1, :])
                    rc16 = eps.tile([1, 2 * QSP], BF16, tag="rc16")
                    nc.vector.tensor_copy(rc16, rc)
                    # broadcast 1/den across 64 partitions per head via two
                    # col-tiled K=1 matmuls
                    rb_ps = pps.tile([P, QSP], F32, tag="pp", name=f"rb_{p}_{sp}")
                    nc.tensor.matmul(
                        rb_ps[0:DH, :], ones_t, rc16[:, 0:QSP],
                        start=True, stop=True,
                    )
                    nc.tensor.matmul(
                        rb_ps[DH:P, :], ones_t, rc16[:, QSP:2 * QSP],
                        start=True, stop=True,
                    )
                    rb = eps.tile([P, QSP], F32, tag="rb")
                    nc.vector.tensor_copy(rb, rb_ps)
                    nc.vector.tensor_mul(oT[:, p, qsl], o_pair, rb)
            # tail: O-projection span 1
            for sc_i in range(QSP // P, Sq // P):
                for spc in range(NCS):
                    o_chunk(sc_i, spc)

    nc.compile()
    return nc


_NC = None


def _get_nc():
    global _NC
    if _NC is None:
        _NC = build_mha_nc(S=S_FULL, Sq=S_FULL // 2, D=D_FULL, H=H_FULL)
    return _NC


def shard_inputs(inputs):
    x = np.asarray(inputs["x"], dtype=np.float32).astype(NPBF16)
    wnames = ("Wq", "Wk", "Wv", "Wo")
    shared = {n: np.ascontiguousarray(
        np.asarray(inputs[n], dtype=np.float32).astype(NPBF16)) for n in wnames}
    shared["bq"] = np.ascontiguousarray(np.asarray(inputs["bq"], dtype=np.float32))
    # bv contributes bv @ Wo to y (attention rows sum to 1); fold into bo
    bv = np.asarray(inputs["bv"], dtype=np.float32)
    Wo = np.asarray(inputs["Wo"], dtype=np.float32)
    bo = np.asarray(inputs["bo"], dtype=np.float32)
    shared["bo"] = np.ascontiguousarray(bo + bv @ Wo)
    half = S_FULL // 2
    maps = []
    for c in range(N_CORES):
        b, h = divmod(c, 2)
        xb = x[b]
        xp = np.concatenate([xb[h * half:(h + 1) * half],
                             xb[(1 - h) * half:(2 - h) * half]], axis=0)
        m = dict(shared)
        m["x"] = np.ascontiguousarray(xp)
        maps.append(m)
    return maps


def run(inputs, trace=False):
    nc = _get_nc()
    maps = shard_inputs(inputs)
    res = run_bass_kernel_spmd(nc, maps, list(range(N_CORES)), trace=trace)
    half = S_FULL // 2
    y = np.empty((B_FULL, S_FULL, D_FULL), dtype=np.float32)
    for c in range(N_CORES):
        b, h = divmod(c, 2)
        y[b, h * half:(h + 1) * half] = res.results[c]["y"]
    return y, res


def kernel(**inputs):
    y, _ = run(inputs, trace=False)
    return y


# revision 7
# speedup vs baseline: 1.8039x; 1.8039x over previous
"""Trainium2 Bass kernel for nn_MultiHeadAttention (B=4, S=2048, D=1024, H=16).

Sharding: 8 cores = 4 batches x 2 query-halves. Each core computes full K/V
projections for its batch (keys are permuted so the core's own queries come
first), attention for its 1024 queries over all 2048 keys, and the output
projection for its query half. No collectives needed.

Per-core dataflow (bf16 matmuls, fp32 PSUM accumulation, all tensors SBUF
resident):
  x [2048,1024] bf16 --PE transpose--> xT [D,S], streamed just-in-time so
  the first scores matmul issues ~9us in (transposes and K/V/Q projections
  are interleaved into the pair-0 attention loop as deferred jobs).
  qT = (x @ Wq + bq)^T per pair; kT = (x @ Wk)^T per pair (bk dropped: the
  q.bk score offset is constant over keys and cancels in softmax).
  v_aug = [x @ Wv | ones] (no bv: it contributes bv@Wo to y since
  sum(attn)=1, folded into bo on the host).
  Per head pair p, per q-span of 512:
    scoresT[k,q] via row-paired (tile_position) K=64 matmuls
    exp on ACT (scale=1/8 folded in), no max subtraction; ACT exp table
    pre-warmed at kernel start.
    attnV with M=65 aug (softmax denominator rides row 64 of the PSUM
    accumulator - the only partition-dim reduction the machine does well).
    normalize by row 64 (reciprocal_approx_fast + K=1 broadcast matmuls).
  y = oT^T @ Wo + bo'  (bo' = bo + bv@Wo precomputed on host)
  O-projection for span 0 interleaved into pair 7 span 1 to shorten the
  tail.
"""

import numpy as np
import ml_dtypes
from contextlib import ExitStack

import concourse.bass as bass
from concourse import bacc
import concourse.mybir as mybir
import concourse.tile as tile
from concourse.bass_utils import run_bass_kernel_spmd
from concourse.masks import make_identity

F32 = mybir.dt.float32
BF16 = mybir.dt.bfloat16
AF = mybir.ActivationFunctionType
NPBF16 = ml_dtypes.bfloat16

P = 128

N_CORES = 8
B_FULL, S_FULL, D_FULL = 4, 2048, 1024
H_FULL, DH = 16, 64


def build_mha_nc(S=2048, Sq=1024, D=1024, H=16, scale=None):
    """Build the per-core Bass program. Returns nc."""
    assert D % P == 0 and S % P == 0 and Sq % P == 0 and H % 2 == 0
    ND = D // P            # d-tiles
    NS = S // P            # s-chunks / k-tiles
    NPAIR = H // 2
    W65 = DH + 1           # augmented head width (v | ones)
    QSP = min(512, Sq)     # q span
    NQS = Sq // QSP
    KSP = min(512, S)      # span for kT projection
    NKS = S // KSP
    CSP = min(512, D)      # col span for v / out projections
    NCS = D // CSP
    HPS = CSP // DH        # heads per col-span in v projection
    if scale is None:
        scale = DH ** -0.5

    nc = bacc.Bacc(target_bir_lowering=False, debug=False)

    x = nc.dram_tensor("x", [S, D], BF16, kind="ExternalInput").ap()
    W = {n: nc.dram_tensor(n, [D, D], BF16, kind="ExternalInput").ap()
         for n in ("Wq", "Wk", "Wv", "Wo")}
    bias = {n: nc.dram_tensor(n, [D], F32, kind="ExternalInput").ap()
            for n in ("bq", "bo")}
    ones_d = nc.dram_tensor("cst_ones", [P, P], BF16, kind="ExternalInput").ap()
    y = nc.dram_tensor("y", [Sq, D], F32, kind="ExternalOutput").ap()

    with tile.TileContext(nc) as tc, ExitStack() as top:
        top.enter_context(nc.allow_low_precision(
            reason="bf16 activations/weights with fp32 psum accumulation"))
        const = top.enter_context(tc.tile_pool(name="const", bufs=1))
        big = top.enter_context(tc.tile_pool(name="big", bufs=1))
        wp = top.enter_context(tc.tile_pool(name="wp", bufs=2))
        kpool = top.enter_context(tc.tile_pool(name="kpool", bufs=3))

        ident = const.tile([P, P], BF16)
        make_identity(nc, ident)
        # bf16 ones row: K=1 stationary broadcasting the softmax reciprocal
        ones_t = const.tile([1, DH], BF16)
        nc.vector.memset(ones_t, 1.0)
        # warm the ACT exp table while DMAs run
        warm = const.tile([1, 2], BF16)
        nc.scalar.activation(warm, ones_t[:, 0:2], AF.Exp, scale=1.0)

        # per-partition bias layouts: b_sb[p, j] = b[j*128 + p]
        bq_sb = const.tile([P, ND], F32)
        nc.gpsimd.dma_start(out=bq_sb, in_=bias["bq"].rearrange("(j p) -> p j", p=P))
        # bo broadcast across partitions (0-stride DRAM read)
        bo_bc = const.tile([P, D], F32)
        nc.gpsimd.dma_start(
            out=bo_bc,
            in_=bias["bo"].unsqueeze(0).partition_broadcast(P).squeeze(1),
        )

        oT = big.tile([P, ND, Sq], BF16)
        xT = big.tile([P, ND, S], BF16)
        qTs = big.tile([P, ND, Sq], BF16)
        v_sb = big.tile([P, NS, H * W65], BF16)
        v3 = v_sb.rearrange("p i (h w) -> p i h w", w=W65)

        # weight staging (wp "w" rotates: Wk, Wq, then Wo; Wv pinned)
        Wk_sb = wp.tile([P, ND, D], BF16, tag="w", name="Wk")
        nc.sync.dma_start(out=Wk_sb, in_=W["Wk"].rearrange("(j p) c -> p j c", p=P))
        Wq_sb = wp.tile([P, ND, D], BF16, tag="w", name="Wq")
        nc.sync.dma_start(out=Wq_sb, in_=W["Wq"].rearrange("(j p) c -> p j c", p=P))
        Wv_sb = wp.tile([P, ND, D], BF16, tag="wv", bufs=1)
        nc.sync.dma_start(out=Wv_sb, in_=W["Wv"].rearrange("(j p) c -> p j c", p=P))

        kps = {}
        wo_box = {}

        with tc.tile_pool(name="xchunk", bufs=3) as xpool, \
             tc.tile_pool(name="tps", bufs=1, space="PSUM") as tpsum, \
             tc.tile_pool(name="pps", bufs=1, space="PSUM") as pps, \
             tc.tile_pool(name="exp", bufs=4) as exq, \
             tc.tile_pool(name="eps", bufs=2) as eps, \
             tc.tile_pool(name="scps", bufs=2, space="PSUM") as scps, \
             tc.tile_pool(name="ops", bufs=2, space="PSUM") as opsum, \
             tc.tile_pool(name="ystg", bufs=2) as ystg:

            # single PSUM bank for transposes, manually double-buffered
            tps2 = tpsum.tile([P, 2, P], BF16)

            def transp(i):
                xc = xpool.tile([P, D], BF16, tag="xc", name=f"xc_{i}")
                nc.sync.dma_start(out=xc, in_=x[i * P:(i + 1) * P, :])
                for j in range(ND):
                    tp = tps2[:, (i * ND + j) % 2, :]
                    nc.tensor.transpose(tp, xc[:, j * P:(j + 1) * P], ident)
                    nc.vector.tensor_copy(xT[:, j, i * P:(i + 1) * P], tp)

            def qT_proj(dc, sp):
                ps = pps.tile([P, QSP], F32, tag="pp", name=f"qps_{dc}_{sp}")
                for j in range(ND):
                    nc.tensor.matmul(
                        ps,
                        Wq_sb[:, j, dc * P:(dc + 1) * P],
                        xT[:, j, sp * QSP:(sp + 1) * QSP],
                        start=(j == 0), stop=(j == ND - 1),
                    )
                nc.vector.tensor_scalar_add(
                    qTs[:, dc, sp * QSP:(sp + 1) * QSP], ps, bq_sb[:, dc:dc + 1])

            def v_proj(i, sp):
                if sp == 0:
                    nc.sync.dma_start(out=v3[:, i, :, DH:DH + 1],
                                      in_=ones_d[:, 0:H].unsqueeze(2))
                ps = pps.tile([P, CSP], F32, tag="pp", name=f"vps_{i}_{sp}")
                for j in range(ND):
                    nc.tensor.matmul(
                        ps,
                        xT[:, j, i * P:(i + 1) * P],
                        Wv_sb[:, j, sp * CSP:(sp + 1) * CSP],
                        start=(j == 0), stop=(j == ND - 1),
                    )
                nc.vector.tensor_copy(
                    v3[:, i, sp * HPS:(sp + 1) * HPS, 0:DH],
                    ps.rearrange("p (h w) -> p h w", w=DH),
                )

            def kT_span(p, sp):
                kp = kps[p]
                ps = pps.tile([P, KSP], F32, tag="pp", name=f"kps_{p}_{sp}")
                for j in range(ND):
                    nc.tensor.matmul(
                        ps,
                        Wk_sb[:, j, p * P:(p + 1) * P],
                        xT[:, j, sp * KSP:(sp + 1) * KSP],
                        start=(j == 0), stop=(j == ND - 1),
                    )
                nc.vector.tensor_copy(kp[:, sp * KSP:(sp + 1) * KSP], ps)

            def new_kp(p):
                kps[p] = kpool.tile([P, S], BF16, tag="kp", name=f"kp_{p}")

            def load_wo():
                Wo_sb = wp.tile([P, ND, D], BF16, tag="w", name="Wo")
                nc.sync.dma_start(
                    out=Wo_sb, in_=W["Wo"].rearrange("(j p) c -> p j c", p=P))
                wo_box["Wo"] = Wo_sb

            def o_chunk(sc_i, spc):
                Wo_sb = wo_box["Wo"]
                ps = pps.tile([P, CSP], F32, tag="pp", name=f"yps_{sc_i}_{spc}")
                for j in range(ND):
                    nc.tensor.matmul(
                        ps,
                        oT[:, j, sc_i * P:(sc_i + 1) * P],
                        Wo_sb[:, j, spc * CSP:(spc + 1) * CSP],
                        start=(j == 0), stop=(j == ND - 1),
                    )
                ysb = ystg.tile([P, CSP], F32, tag="ysb")
                nc.vector.tensor_add(ysb, ps, bo_bc[:, spc * CSP:(spc + 1) * CSP])
                nc.sync.dma_start(
                    out=y[sc_i * P:(sc_i + 1) * P, spc * CSP:(spc + 1) * CSP],
                    in_=ysb,
                )

            # ---- slim prologue: just enough for pair 0 span 0 kt 0-3 ----
            for i in range(4):
                transp(i)
            new_kp(0)
            kT_span(0, 0)
            qT_proj(0, 0)

            # deferred-work schedule: (pair, span, kt) -> [thunks]
            jobs = {}

            def add(p, sp, kt, fn):
                jobs.setdefault((p, sp, kt), []).append(fn)

            # pair 0, span 0: remaining transposes + kp(0) spans (v is jit)
            add(0, 0, 1, lambda: (transp(4), transp(5)))
            add(0, 0, 2, lambda: (transp(6), transp(7)))
            add(0, 0, 3, lambda: kT_span(0, 1))
            add(0, 0, 4, lambda: (transp(8), transp(9)))
            add(0, 0, 5, lambda: (transp(10), transp(11)))
            add(0, 0, 6, lambda: kT_span(0, 2))
            add(0, 0, 7, lambda: (transp(12), transp(13)))
            add(0, 0, 8, lambda: (transp(14), transp(15)))
            add(0, 0, 9, lambda: kT_span(0, 3))

            # pair 0, span 1: qT(0,1), kp(1), qT(1)
            add(0, 1, 0, lambda: qT_proj(0, 1))
            add(0, 1, 1, lambda: new_kp(1))
            for sp_ in range(NKS):
                add(0, 1, 2 + sp_ * 2, lambda sp_=sp_: kT_span(1, sp_))
            add(0, 1, 11, lambda: qT_proj(1, 0))
            add(0, 1, 13, lambda: qT_proj(1, 1))

            # pairs 1..6: kp(p+1) in span 0, qT(p+1) in span 1
            for p_ in range(1, NPAIR - 1):
                add(p_, 0, 0, lambda p_=p_: new_kp(p_ + 1))
                for sp_ in range(NKS):
                    add(p_, 0, 1 + sp_ * 3,
                        lambda p_=p_, sp_=sp_: kT_span(p_ + 1, sp_))
                add(p_, 1, 1, lambda p_=p_: qT_proj(p_ + 1, 0))
                add(p_, 1, 8, lambda p_=p_: qT_proj(p_ + 1, 1))
            # v sp1 (heads 8-15, needed from pair 4): 16 chains over pairs 1-3
            vslots = [(p_, sp_, kt_) for p_ in (1, 2, 3)
                      for sp_ in (0, 1) for kt_ in (2, 6, 12)]
            for i_ in range(NS):
                p_, sp_, kt_ = vslots[i_]
                add(p_, sp_, kt_, lambda i_=i_: v_proj(i_, 1))
            # Wo load + O-projection span 0 interleaved into pair 7
            add(NPAIR - 2, 1, 14, load_wo)
            for ci, (sc_i, spc) in enumerate(
                    [(si, c) for si in range(QSP // P) for c in range(NCS)]):
                add(NPAIR - 1, 1, 2 * ci, lambda a=sc_i, b=spc: o_chunk(a, b))

            # ---- attention: pair-outer, span-inner ----
            for p in range(NPAIR):
                for sp in range(NQS):
                    qsl = slice(sp * QSP, (sp + 1) * QSP)
                    kp = kps[p]
                    o_even = opsum.tile([W65, QSP], F32, tag="op")
                    o_odd = opsum.tile([W65, QSP], F32, tag="op")
                    for kt in range(NS):
                        for fn in jobs.get((p, sp, kt), ()):
                            fn()
                        sc = scps.tile([P, 2 * QSP], F32, tag="sc")
                        nc.tensor.matmul(
                            sc[:, 0:QSP],
                            kp[0:DH, kt * P:(kt + 1) * P],
                            qTs[0:DH, p, qsl],
                            start=True, stop=True,
                        )
                        nc.tensor.matmul(
                            sc[:, QSP:2 * QSP],
                            kp[DH:P, kt * P:(kt + 1) * P],
                            qTs[DH:P, p, qsl],
                            start=True, stop=True,
                        )
                        ex = exq.tile([P, 2 * QSP], BF16, tag="ex")
                        nc.scalar.activation(ex, sc, AF.Exp, scale=float(scale))
                        if p == 0 and sp == 0:
                            v_proj(kt, 0)
                        nc.tensor.matmul(
                            o_even,
                            v3[:, kt, 2 * p, :],
                            ex[:, 0:QSP],
                            start=(kt == 0), stop=(kt == NS - 1),
                        )
                        nc.tensor.matmul(
                            o_odd,
                            v3[:, kt, 2 * p + 1, :],
                            ex[:, QSP:2 * QSP],
                            start=(kt == 0), stop=(kt == NS - 1),
                        )
                    # epilogue: denominators ride row 64 of o_even/o_odd;
                    # one wide reciprocal, two col-tiled K=1 broadcast
                    # matmuls, two normalize-multiplies into oT
                    den = eps.tile([1, 2 * QSP], F32, tag="den")
                    nc.vector.tensor_copy(den[:, 0:QSP], o_even[DH:W65, :])
                    nc.vector.tensor_copy(den[:, QSP:2 * QSP], o_odd[DH:W65, :])
                    rc = eps.tile([1, 2 * QSP], F32, tag="rc")
                    nc.vector.reciprocal_approx_fast(rc, den)
                    rc16 = eps.tile([1, 2 * QSP], BF16, tag="rc16")
                    nc.vector.tensor_copy(rc16, rc)
                    rb_ps = pps.tile([P, QSP], F32, tag="pp", name=f"rb_{p}_{sp}")
                    nc.tensor.matmul(
                        rb_ps[0:DH, :], ones_t, rc16[:, 0:QSP],
                        start=True, stop=True,
                    )
                    nc.tensor.matmul(
                        rb_ps[DH:P, :], ones_t, rc16[:, QSP:2 * QSP],
                        start=True, stop=True,
                    )
                    rb = eps.tile([P, QSP], F32, tag="rb")
                    nc.vector.tensor_copy(rb, rb_ps)
                    nc.vector.tensor_mul(oT[0:DH, p, qsl], o_even[0:DH, :],
                                         rb[0:DH, :])
                    nc.vector.tensor_mul(oT[DH:P, p, qsl], o_odd[0:DH, :],
                                         rb[DH:P, :])
            # tail: O-projection span 1
            for sc_i in range(QSP // P, Sq // P):
                for spc in range(NCS):
                    o_chunk(sc_i, spc)

    nc.compile()
    return nc


_NC = None


def _get_nc():
    global _NC
    if _NC is None:
        _NC = build_mha_nc(S=S_FULL, Sq=S_FULL // 2, D=D_FULL, H=H_FULL)
    return _NC


def shard_inputs(inputs):
    x = np.asarray(inputs["x"], dtype=np.float32).astype(NPBF16)
    wnames = ("Wq", "Wk", "Wv", "Wo")
    shared = {n: np.ascontiguousarray(
        np.asarray(inputs[n], dtype=np.float32).astype(NPBF16)) for n in wnames}
    shared["bq"] = np.ascontiguousarray(np.asarray(inputs["bq"], dtype=np.float32))
    # bv contributes bv @ Wo to y (attention rows sum to 1); fold into bo
    bv = np.asarray(inputs["bv"], dtype=np.float32)
    Wo = np.asarray(inputs["Wo"], dtype=np.float32)
    bo = np.asarray(inputs["bo"], dtype=np.float32)
    shared["bo"] = np.ascontiguousarray(bo + bv @ Wo)
    shared["cst_ones"] = np.ones((P, P), dtype=NPBF16)
    half = S_FULL // 2
    maps = []
    for c in range(N_CORES):
        b, h = divmod(c, 2)
        xb = x[b]
        xp = np.concatenate([xb[h * half:(h + 1) * half],
                             xb[(1 - h) * half:(2 - h) * half]], axis=0)
        m = dict(shared)
        m["x"] = np.ascontiguousarray(xp)
        maps.append(m)
    return maps


def run(inputs, trace=False):
    nc = _get_nc()
    maps = shard_inputs(inputs)
    res = run_bass_kernel_spmd(nc, maps, list(range(N_CORES)), trace=trace)
    half = S_FULL // 2
    y = np.empty((B_FULL, S_FULL, D_FULL), dtype=np.float32)
    for c in range(N_CORES):
        b, h = divmod(c, 2)
        y[b, h * half:(h + 1) * half] = res.results[c]["y"]
    return y, res


def kernel(**inputs):
    y, _ = run(inputs, trace=False)
    return y


# revision 10
# speedup vs baseline: 2.0033x; 1.1105x over previous
"""Trainium2 Bass kernel for nn_MultiHeadAttention (B=4, S=2048, D=1024, H=16).

Sharding: 8 cores = 4 batches x 2 query-halves. Each core computes full K/V
projections for its batch (keys are permuted so the core's own queries come
first), attention for its 1024 queries over all 2048 keys, and the output
projection for its query half. No collectives needed.

Per-core dataflow (bf16 matmuls, fp32 PSUM accumulation, all tensors SBUF
resident):
  x [2048,1024] bf16 --PE transpose--> xT [D,S] in a short prologue whose
  PSUM pool is closed before the attention pools open (PSUM bank budget).
  qT = (x @ Wq + bq)^T per pair; kT = (x @ Wk)^T per pair (bk dropped: the
  q.bk score offset is constant over keys and cancels in softmax).
  v_aug = [x @ Wv | ones] (no bv: it contributes bv@Wo to y since
  sum(attn)=1, folded into bo on the host). K/Q/V projections beyond what
  pair 0 needs are interleaved into the attention loop as deferred jobs.
  Per head pair p, per q-span of 512:
    scoresT[k,q] via row-paired (tile_position) K=64 matmuls
    exp on ACT (scale=1/8 folded in), no max subtraction; ACT exp table
    pre-warmed at kernel start.
    attnV with M=65 aug (softmax denominator rides row 64 of the PSUM
    accumulator - the only partition-dim reduction the machine does well).
    epilogue is split: unnormalized rows and denominators are staged out of
    PSUM immediately (frees the accumulators for the next span); the
    reciprocal + K=1 broadcast matmuls + in-place normalize of oT are
    deferred into the next span so the PE never waits on the DVE chain.
  y = oT^T @ Wo + bo'  (bo' = bo + bv@Wo precomputed on host)
  O-projection for span 0 interleaved into pair 7 span 1 to shorten the
  tail.
"""

import numpy as np
import ml_dtypes
from contextlib import ExitStack

import concourse.bass as bass
from concourse import bacc
import concourse.mybir as mybir
import concourse.tile as tile
from concourse.bass_utils import run_bass_kernel_spmd
from concourse.masks import make_identity

F32 = mybir.dt.float32
BF16 = mybir.dt.bfloat16
AF = mybir.ActivationFunctionType
NPBF16 = ml_dtypes.bfloat16

P = 128

N_CORES = 8
B_FULL, S_FULL, D_FULL = 4, 2048, 1024
H_FULL, DH = 16, 64


def build_mha_nc(S=2048, Sq=1024, D=1024, H=16, scale=None):
    """Build the per-core Bass program. Returns nc."""
    assert D % P == 0 and S % P == 0 and Sq % P == 0 and H % 2 == 0
    ND = D // P            # d-tiles
    NS = S // P            # s-chunks / k-tiles
    NPAIR = H // 2
    W65 = DH + 1           # augmented head width (v | ones)
    QSP = min(512, Sq)     # q span
    NQS = Sq // QSP
    KSP = min(512, S)      # span for kT projection
    NKS = S // KSP
    CSP = min(512, D)      # col span for v / out projections
    NCS = D // CSP
    HPS = CSP // DH        # heads per col-span in v projection
    if scale is None:
        scale = DH ** -0.5

    nc = bacc.Bacc(target_bir_lowering=False, debug=False)

    x = nc.dram_tensor("x", [S, D], BF16, kind="ExternalInput").ap()
    W = {n: nc.dram_tensor(n, [D, D], BF16, kind="ExternalInput").ap()
         for n in ("Wq", "Wk", "Wv", "Wo")}
    bias = {n: nc.dram_tensor(n, [D], F32, kind="ExternalInput").ap()
            for n in ("bq", "bo")}
    ones_d = nc.dram_tensor("cst_ones", [P, P], BF16, kind="ExternalInput").ap()
    y = nc.dram_tensor("y", [Sq, D], F32, kind="ExternalOutput").ap()

    with tile.TileContext(nc) as tc, ExitStack() as top:
        top.enter_context(nc.allow_low_precision(
            reason="bf16 activations/weights with fp32 psum accumulation"))
        const = top.enter_context(tc.tile_pool(name="const", bufs=1))
        big = top.enter_context(tc.tile_pool(name="big", bufs=1))
        wp = top.enter_context(tc.tile_pool(name="wp", bufs=2))
        kpool = top.enter_context(tc.tile_pool(name="kpool", bufs=3))

        ident = const.tile([P, P], BF16)
        make_identity(nc, ident)
        # bf16 ones row: K=1 stationary broadcasting the softmax reciprocal
        ones_t = const.tile([1, DH], BF16)
        nc.vector.memset(ones_t, 1.0)
        # warm the ACT exp table while DMAs run
        warm = const.tile([1, 2], BF16)
        nc.scalar.activation(warm, ones_t[:, 0:2], AF.Exp, scale=1.0)

        # per-partition bias layouts: b_sb[p, j] = b[j*128 + p]
        bq_sb = const.tile([P, ND], F32)
        nc.gpsimd.dma_start(out=bq_sb, in_=bias["bq"].rearrange("(j p) -> p j", p=P))
        # bo broadcast across partitions (0-stride DRAM read)
        bo_bc = const.tile([P, D], F32)
        nc.gpsimd.dma_start(
            out=bo_bc,
            in_=bias["bo"].unsqueeze(0).partition_broadcast(P).squeeze(1),
        )

        oT = big.tile([P, ND, Sq], BF16)
        xT = big.tile([P, ND, S], BF16)
        qTs = big.tile([P, ND, Sq], BF16)
        v_sb = big.tile([P, NS, H * W65], BF16)
        v3 = v_sb.rearrange("p i (h w) -> p i h w", w=W65)

        # weight staging (wp "w" rotates: Wk, Wq, then Wo; Wv pinned)
        Wk_sb = wp.tile([P, ND, D], BF16, tag="w", name="Wk")
        nc.sync.dma_start(out=Wk_sb, in_=W["Wk"].rearrange("(j p) c -> p j c", p=P))
        Wq_sb = wp.tile([P, ND, D], BF16, tag="w", name="Wq")
        nc.sync.dma_start(out=Wq_sb, in_=W["Wq"].rearrange("(j p) c -> p j c", p=P))
        Wv_sb = wp.tile([P, ND, D], BF16, tag="wv", bufs=1)
        nc.sync.dma_start(out=Wv_sb, in_=W["Wv"].rearrange("(j p) c -> p j c", p=P))

        kps = {}
        wo_box = {}

        with tc.tile_pool(name="exp", bufs=4) as exq, \
             tc.tile_pool(name="eps", bufs=2) as eps, \
             tc.tile_pool(name="scps", bufs=2, space="PSUM") as scps, \
             tc.tile_pool(name="ystg", bufs=2) as ystg:

            # ---- prologue: transposes use a scoped 2-bank PSUM pool that
            # is closed before the attention pools open ----
            with tc.tile_pool(name="xchunk", bufs=3) as xpool, \
                 tc.tile_pool(name="tps", bufs=2, space="PSUM") as tpsum, \
                 tc.tile_pool(name="ppE", bufs=2, space="PSUM") as ppE:
                for i in range(NS):
                    xc = xpool.tile([P, D], BF16, tag="xc", name=f"xc_{i}")
                    nc.sync.dma_start(out=xc, in_=x[i * P:(i + 1) * P, :])
                    for j in range(ND):
                        tp = tpsum.tile([P, P], BF16, tag="tp")
                        nc.tensor.transpose(tp, xc[:, j * P:(j + 1) * P], ident)
                        nc.vector.tensor_copy(xT[:, j, i * P:(i + 1) * P], tp)
                # first-pair prerequisites, using the prologue PSUM pool
                kps[0] = kpool.tile([P, S], BF16, tag="kp", name="kp_0")
                for sp_ in range(NKS):
                    ps = ppE.tile([P, KSP], F32, tag="pp", name=f"kpsE_{sp_}")
                    for j in range(ND):
                        nc.tensor.matmul(
                            ps,
                            Wk_sb[:, j, 0:P],
                            xT[:, j, sp_ * KSP:(sp_ + 1) * KSP],
                            start=(j == 0), stop=(j == ND - 1),
                        )
                    nc.vector.tensor_copy(
                        kps[0][:, sp_ * KSP:(sp_ + 1) * KSP], ps)
                ps = ppE.tile([P, QSP], F32, tag="pp", name="qpsE")
                for j in range(ND):
                    nc.tensor.matmul(
                        ps, Wq_sb[:, j, 0:P], xT[:, j, 0:QSP],
                        start=(j == 0), stop=(j == ND - 1),
                    )
                nc.vector.tensor_scalar_add(
                    qTs[:, 0, 0:QSP], ps, bq_sb[:, 0:1])

            with tc.tile_pool(name="pps", bufs=2, space="PSUM") as pps, \
                 tc.tile_pool(name="ops", bufs=2, space="PSUM") as opsum:

                def qT_proj(dc, sp):
                    ps = pps.tile([P, QSP], F32, tag="pp", name=f"qps_{dc}_{sp}")
                    for j in range(ND):
                        nc.tensor.matmul(
                            ps,
                            Wq_sb[:, j, dc * P:(dc + 1) * P],
                            xT[:, j, sp * QSP:(sp + 1) * QSP],
                            start=(j == 0), stop=(j == ND - 1),
                        )
                    nc.vector.tensor_scalar_add(
                        qTs[:, dc, sp * QSP:(sp + 1) * QSP], ps,
                        bq_sb[:, dc:dc + 1])

                def v_proj(i, sp):
                    if sp == 0:
                        nc.sync.dma_start(out=v3[:, i, :, DH:DH + 1],
                                          in_=ones_d[:, 0:H].unsqueeze(2))
                    ps = pps.tile([P, CSP], F32, tag="pp", name=f"vps_{i}_{sp}")
                    for j in range(ND):
                        nc.tensor.matmul(
                            ps,
                            xT[:, j, i * P:(i + 1) * P],
                            Wv_sb[:, j, sp * CSP:(sp + 1) * CSP],
                            start=(j == 0), stop=(j == ND - 1),
                        )
                    nc.vector.tensor_copy(
                        v3[:, i, sp * HPS:(sp + 1) * HPS, 0:DH],
                        ps.rearrange("p (h w) -> p h w", w=DH),
                    )

                def kT_span(p, sp):
                    kp = kps[p]
                    ps = pps.tile([P, KSP], F32, tag="pp", name=f"kps_{p}_{sp}")
                    for j in range(ND):
                        nc.tensor.matmul(
                            ps,
                            Wk_sb[:, j, p * P:(p + 1) * P],
                            xT[:, j, sp * KSP:(sp + 1) * KSP],
                            start=(j == 0), stop=(j == ND - 1),
                        )
                    nc.vector.tensor_copy(kp[:, sp * KSP:(sp + 1) * KSP], ps)

                def new_kp(p):
                    kps[p] = kpool.tile([P, S], BF16, tag="kp", name=f"kp_{p}")

                def load_wo():
                    Wo_sb = wp.tile([P, ND, D], BF16, tag="w", name="Wo")
                    nc.sync.dma_start(
                        out=Wo_sb, in_=W["Wo"].rearrange("(j p) c -> p j c", p=P))
                    wo_box["Wo"] = Wo_sb

                def o_chunk(sc_i, spc):
                    Wo_sb = wo_box["Wo"]
                    ps = pps.tile([P, CSP], F32, tag="pp",
                                  name=f"yps_{sc_i}_{spc}")
                    for j in range(ND):
                        nc.tensor.matmul(
                            ps,
                            oT[:, j, sc_i * P:(sc_i + 1) * P],
                            Wo_sb[:, j, spc * CSP:(spc + 1) * CSP],
                            start=(j == 0), stop=(j == ND - 1),
                        )
                    ysb = ystg.tile([P, CSP], F32, tag="ysb")
                    nc.vector.tensor_add(
                        ysb, ps, bo_bc[:, spc * CSP:(spc + 1) * CSP])
                    nc.sync.dma_start(
                        out=y[sc_i * P:(sc_i + 1) * P,
                              spc * CSP:(spc + 1) * CSP],
                        in_=ysb,
                    )

                # deferred-work schedule: (pair, span, kt) -> [thunks]
                jobs = {}

                def add(p, sp, kt, fn):
                    jobs.setdefault((p, sp, kt), []).append(fn)

                # pair 0: v sp0 is jit in the kt loop; qT(0,1), kp(1), qT(1)
                add(0, 1, 0, lambda: qT_proj(0, 1))
                add(0, 1, 1, lambda: new_kp(1))
                for sp_ in range(NKS):
                    add(0, 1, 3 + sp_ * 2, lambda sp_=sp_: kT_span(1, sp_))
                add(0, 1, 11, lambda: qT_proj(1, 0))
                add(0, 1, 13, lambda: qT_proj(1, 1))

                # pairs 1..6: kp(p+1) in span 0, qT(p+1) in span 1
                for p_ in range(1, NPAIR - 1):
                    add(p_, 0, 0, lambda p_=p_: new_kp(p_ + 1))
                    for sp_ in range(NKS):
                        add(p_, 0, 3 + sp_ * 3,
                            lambda p_=p_, sp_=sp_: kT_span(p_ + 1, sp_))
                    add(p_, 1, 3, lambda p_=p_: qT_proj(p_ + 1, 0))
                    add(p_, 1, 8, lambda p_=p_: qT_proj(p_ + 1, 1))
                # v sp1 (heads 8-15, needed from pair 4) over pairs 1-3
                vslots = [(p_, sp_, kt_) for p_ in (1, 2, 3)
                          for sp_ in (0, 1) for kt_ in (5, 10, 14)]
                for i_ in range(NS):
                    p_, sp_, kt_ = vslots[i_]
                    add(p_, sp_, kt_, lambda i_=i_: v_proj(i_, 1))
                # Wo load + O-projection span 0 interleaved into pair 7
                add(NPAIR - 2, 1, 14, load_wo)
                # slots start after the deferred epilogue of (7, span 0)
                # fires at kt==2, so oT pair-7 columns are normalized first
                for (sc_i, spc), kt_ in zip(
                        [(si, c) for si in range(QSP // P) for c in range(NCS)],
                        (3, 4, 6, 8, 10, 12, 14, 15)):
                    add(NPAIR - 1, 1, kt_, lambda a=sc_i, b=spc: o_chunk(a, b))

                # deferred epilogue part 2: reciprocal + broadcast + in-place
                # normalize of oT; runs inside the NEXT span's kt loop
                def epi_b(p, sp, den):
                    rc = eps.tile([1, 2 * QSP], F32, tag="rc")
                    nc.vector.reciprocal_approx_fast(rc, den)
                    rc16 = eps.tile([1, 2 * QSP], BF16, tag="rc16")
                    nc.vector.tensor_copy(rc16, rc)
                    qsl = slice(sp * QSP, (sp + 1) * QSP)
                    rb_ps = pps.tile([P, QSP], F32, tag="pp",
                                     name=f"rb_{p}_{sp}")
                    nc.tensor.matmul(
                        rb_ps[0:DH, :], ones_t, rc16[:, 0:QSP],
                        start=True, stop=True,
                    )
                    nc.tensor.matmul(
                        rb_ps[DH:P, :], ones_t, rc16[:, QSP:2 * QSP],
                        start=True, stop=True,
                    )
                    rb = eps.tile([P, QSP], F32, tag="rb")
                    nc.vector.tensor_copy(rb, rb_ps)
                    nc.vector.tensor_mul(oT[:, p, qsl], oT[:, p, qsl], rb)

                pending = []

                # ---- attention: pair-outer, span-inner ----
                for p in range(NPAIR):
                    for sp in range(NQS):
                        qsl = slice(sp * QSP, (sp + 1) * QSP)
                        kp = kps[p]
                        o_even = opsum.tile([W65, QSP], F32, tag="op")
                        o_odd = opsum.tile([W65, QSP], F32, tag="op")
                        for kt in range(NS):
                            if kt == 2 and pending:
                                pending.pop()()
                            for fn in jobs.get((p, sp, kt), ()):
                                fn()
                            sc = scps.tile([P, 2 * QSP], F32, tag="sc")
                            nc.tensor.matmul(
                                sc[:, 0:QSP],
                                kp[0:DH, kt * P:(kt + 1) * P],
                                qTs[0:DH, p, qsl],
                                start=True, stop=True,
                            )
                            nc.tensor.matmul(
                                sc[:, QSP:2 * QSP],
                                kp[DH:P, kt * P:(kt + 1) * P],
                                qTs[DH:P, p, qsl],
                                start=True, stop=True,
                            )
                            ex = exq.tile([P, 2 * QSP], BF16, tag="ex")
                            nc.scalar.activation(ex, sc, AF.Exp,
                                                 scale=float(scale))
                            if p == 0 and sp == 0:
                                v_proj(kt, 0)
                            nc.tensor.matmul(
                                o_even,
                                v3[:, kt, 2 * p, :],
                                ex[:, 0:QSP],
                                start=(kt == 0), stop=(kt == NS - 1),
                            )
                            nc.tensor.matmul(
                                o_odd,
                                v3[:, kt, 2 * p + 1, :],
                                ex[:, QSP:2 * QSP],
                                start=(kt == 0), stop=(kt == NS - 1),
                            )
                        # epilogue part 1: stage denominators and raw rows
                        # out of PSUM so the accumulators free quickly
                        den = eps.tile([1, 2 * QSP], F32, tag="den")
                        nc.vector.tensor_copy(den[:, 0:QSP], o_even[DH:W65, :])
                        nc.vector.tensor_copy(den[:, QSP:2 * QSP],
                                              o_odd[DH:W65, :])
                        nc.vector.tensor_copy(oT[0:DH, p, qsl], o_even[0:DH, :])
                        nc.vector.tensor_copy(oT[DH:P, p, qsl], o_odd[0:DH, :])
                        pending.append(
                            lambda p=p, sp=sp, den=den: epi_b(p, sp, den))
                # flush the last deferred epilogue, then tail O-projection
                while pending:
                    pending.pop()()
                for sc_i in range(QSP // P, Sq // P):
                    for spc in range(NCS):
                        o_chunk(sc_i, spc)

    nc.compile()
    return nc


_NC = None


def _get_nc():
    global _NC
    if _NC is None:
        _NC = build_mha_nc(S=S_FULL, Sq=S_FULL // 2, D=D_FULL, H=H_FULL)
    return _NC


def shard_inputs(inputs):
    x = np.asarray(inputs["x"], dtype=np.float32).astype(NPBF16)
    wnames = ("Wq", "Wk", "Wv", "Wo")
    shared = {n: np.ascontiguousarray(
        np.asarray(inputs[n], dtype=np.float32).astype(NPBF16)) for n in wnames}
    shared["bq"] = np.ascontiguousarray(np.asarray(inputs["bq"], dtype=np.float32))
    # bv contributes bv @ Wo to y (attention rows sum to 1); fold into bo
    bv = np.asarray(inputs["bv"], dtype=np.float32)
    Wo = np.asarray(inputs["Wo"], dtype=np.float32)
    bo = np.asarray(inputs["bo"], dtype=np.float32)
    shared["bo"] = np.ascontiguousarray(bo + bv @ Wo)
    shared["cst_ones"] = np.ones((P, P), dtype=NPBF16)
    half = S_FULL // 2
    maps = []
    for c in range(N_CORES):
        b, h = divmod(c, 2)
        xb = x[b]
        xp = np.concatenate([xb[h * half:(h + 1) * half],
                             xb[(1 - h) * half:(2 - h) * half]], axis=0)
        m = dict(shared)
        m["x"] = np.ascontiguousarray(xp)
        maps.append(m)
    return maps


def run(inputs, trace=False):
    nc = _get_nc()
    maps = shard_inputs(inputs)
    res = run_bass_kernel_spmd(nc, maps, list(range(N_CORES)), trace=trace)
    half = S_FULL // 2
    y = np.empty((B_FULL, S_FULL, D_FULL), dtype=np.float32)
    for c in range(N_CORES):
        b, h = divmod(c, 2)
        y[b, h * half:(h + 1) * half] = res.results[c]["y"]
    return y, res


def kernel(**inputs):
    y, _ = run(inputs, trace=False)
    return y
